# revision 1
# baseline (speedup 1.0000x reference)
"""Trainium2 Bass kernel for a NetVLAD-style VQ codebook module (v2).

reference semantics (B=16, N=2048, D=1024, K=64):
    x = l2norm(grids, axis=D)
    logits = x @ W.T + b            # [B, N, K]
    a = softmax(logits, axis=K)
    p = einsum('bnk,bnd->bkd', a, x) - centroids * a.sum(n)
    out = l2norm(p, axis=D)

Sharding: data-parallel over batch B across 8 cores (2 batches/core).

v2 design (fp8 + DoubleRow + norm-invariant scaling):
  - G fed twice (gt [d,n] + gn [n,d]) in fp8e4m3: 8MB/core DMA (the floor).
  - Row norms s = ||g||^2 via PE fp8-DR Gram tiles; diag extracted with a
    DVE ident-mask STT (accum_out), r' = quake-rsqrt(s*2^10) = r/32.
  - mm1 in [n,k]-out orientation, fp8 DoubleRow (256-deep steps); W*32 fp8.
  - softmax: e = exp(r'*z') fp16 (z' = 32z); e~ = e*exp(b) via DVE STT with
    rank-1-broadcast exp(b), accum -> esum (exact bias handling).
  - a' split (fp8 subnormal dodge): a'*2^15 = rsc + delta, delta = (e~-1)*rsc
    stored fp8 (~0.5 scale), rsc = recip*r*2^15 ~ 16.  Norm-invariance of the
    final l2norm makes the global 2^15 scale free.
  - mm2 in [d,k]-out orientation fp8 DR: p1T[d,k] = sum gn-pair.T @ delta-pair;
    + rank-1 v-broadcast (v = sum_n rsc*g via 1-row matmuls)
    + centT @ diag(-asum*2^9) with cent*2^6 fp16 (total 2^15) -- all in one
    PSUM accumulation group.
  - finalize: qq = Square(p1*2^-10) on ACT; n2 row via ones-matmul; quake rn;
    rank-1 rn broadcast; final DVE STT (p1*2^-10)*rnb -> fp16 out.
  - Emission phases A/B/M/C/D interleaved across the two batches so neither
    engine FIFO blocks the other batch's critical chain.
"""

import os
import sys

sys.path.insert(0, "/opt/trn_rl_repo")

import numpy as np

import concourse.bacc as bacc
import concourse.bass as bass
import concourse.mybir as mybir
import concourse.tile as tile
from concourse.masks import make_identity

B, N, D, K = 16, 2048, 1024, 64
N_CORES = 8
LB = B // N_CORES
NT = N // 128              # 16 n-tiles
DC = 8                     # 8 d-chunks
GW = 4                     # gram wave size (tiles per PSUM wave)

F32 = mybir.dt.float32
F16 = mybir.dt.float16
F8 = mybir.dt.float8e4
I32 = mybir.dt.int32
ALU = mybir.AluOpType
ACTF = mybir.ActivationFunctionType
PM = mybir.MatmulPerfMode

RSQRT_MAGIC = 0x5F3759DF

_CACHE = {}


def _emit_rsqrt(nc, pool, y, x, scratch_shape, iters=2, tag="rsq", eng=None):
    """y = x**-0.5 via quake-seed + Newton (DVE by default, or gpsimd)."""
    e = eng if eng is not None else nc.vector
    t = pool.tile(scratch_shape, F32, tag=tag + "_t")
    u = pool.tile(scratch_shape, F32, tag=tag + "_u")
    e.tensor_scalar(
        out=t.bitcast(I32), in0=x.bitcast(I32),
        scalar1=1, scalar2=None, op0=ALU.arith_shift_right)
    e.tensor_scalar(
        out=y.bitcast(I32), in0=t.bitcast(I32),
        scalar1=-1, scalar2=None, op0=ALU.bitwise_xor)
    e.tensor_scalar(
        out=y.bitcast(I32), in0=y.bitcast(I32),
        scalar1=RSQRT_MAGIC + 1, scalar2=None, op0=ALU.add)
    for _ in range(iters):
        e.tensor_mul(t, y, y)
        e.tensor_mul(u, t, x)
        e.tensor_scalar(
            out=u, in0=u, scalar1=-0.5, scalar2=1.5, op0=ALU.mult, op1=ALU.add)
        e.tensor_mul(y, y, u)


def _gen_nc():
    GRP = int(os.environ.get("K_GRP", "8"))       # softmax group size
    NG = NT // GRP

    nc = bacc.Bacc(None, target_bir_lowering=False)

    gt_d = nc.dram_tensor("gt", [LB, DC, 128, N], F8, kind="ExternalInput")
    gn_d = nc.dram_tensor("gn", [LB, 128, NT, D], F8, kind="ExternalInput")
    wt_d = nc.dram_tensor("wt", [DC, 128, K], F8, kind="ExternalInput")
    bias_d = nc.dram_tensor("bias", [1, K], F32, kind="ExternalInput")
    cent_d = nc.dram_tensor("cent", [K, D], F16, kind="ExternalInput")
    out_d = nc.dram_tensor("out", [LB, 128, DC, K], F16, kind="ExternalOutput")

    from contextlib import ExitStack

    with tile.TileContext(nc) as tc, ExitStack() as ctx:
        singles = ctx.enter_context(tc.tile_pool(name="singles", bufs=1))
        gpool = ctx.enter_context(tc.tile_pool(name="gpool", bufs=2))
        work = ctx.enter_context(tc.tile_pool(name="work", bufs=2))
        scr = ctx.enter_context(tc.tile_pool(name="scr", bufs=2))
        ps_gram = ctx.enter_context(tc.tile_pool(name="ps_gram", bufs=2, space="PSUM"))
        ps_z = ctx.enter_context(tc.tile_pool(name="ps_z", bufs=1, space="PSUM"))
        ps_p1 = ctx.enter_context(tc.tile_pool(name="ps_p1", bufs=2, space="PSUM"))
        ps_small = ctx.enter_context(tc.tile_pool(name="ps_small", bufs=2, space="PSUM"))

        # ---- one-time constants (wt DMA first; bias/cent woven in later) ----
        wt_sb = singles.tile([128, DC, K], F8)
        nc.sync.dma_start(out=wt_sb, in_=wt_d.rearrange("c p k -> p c k"))
        bias_sb = singles.tile([1, K], F32)
        cent_sb = singles.tile([K, D], F16)

        identF = singles.tile([128, 128], F32)
        make_identity(nc, identF)
        ident16 = singles.tile([128, 128], F16)
        make_identity(nc, ident16)
        ident64 = singles.tile([K, K], F16)
        make_identity(nc, ident64)
        ones128 = singles.tile([128, 1], F16)
        nc.vector.memset(ones128, 1.0)
        ones1p = singles.tile([1, 128], F16)
        nc.vector.memset(ones1p, 1.0)
        ones64p = singles.tile([K, 128], F16)
        nc.vector.memset(ones64p, 1.0)

        # ---- input DMA emission ----
        # gt per batch: n-half-major (so first-half grams/mm1 start early),
        # c-pairs inner.  gn per batch: two n-halves.
        gt_sbs = [gpool.tile([128, DC, N], F8, name=f"gt_sb{lb}", tag="gt")
                  for lb in range(LB)]
        gn_sbs = [gpool.tile([128, NT, D], F8, name=f"gn_sb{lb}", tag="gn")
                  for lb in range(LB)]

        def emit_gt_dma(lb):
            for h in range(2):
                ns = slice(h * (N // 2), (h + 1) * (N // 2))
                nc.sync.dma_start(
                    out=gt_sbs[lb][:, :, ns],
                    in_=gt_d[lb, :, :, ns].rearrange("c p n -> p c n"))

        def emit_gn_dma(lb, h, q=1):
            step = NT // 2 // q
            for i in range(q):
                ts = slice(h * (NT // 2) + i * step,
                           h * (NT // 2) + (i + 1) * step)
                nc.sync.dma_start(
                    out=gn_sbs[lb][:, ts, :],
                    in_=gn_d[lb, :, ts, :])

        nc.scalar.dma_start(out=bias_sb, in_=bias_d[:])
        nc.scalar.dma_start(out=cent_sb, in_=cent_d[:])
        emit_gt_dma(0)
        emit_gt_dma(1)
        emit_gn_dma(0, 0)
        emit_gn_dma(1, 0)
        emit_gn_dma(0, 1)
        emit_gn_dma(1, 1, q=2)

        # exp(b) broadcast to all partitions (exact bias handling)
        eb16 = singles.tile([1, K], F16)
        nc.scalar.activation(out=eb16, in_=bias_sb, func=ACTF.Exp)
        ebb_ps = ps_small.tile([128, 512], F32, tag="small", name="ebb_small")
        nc.tensor.matmul(ebb_ps[:, 0:K], ones1p[:, :], eb16[:, :],
                         start=True, stop=True)
        ebb16 = singles.tile([128, K], F16)
        nc.vector.tensor_copy(out=ebb16, in_=ebb_ps[:, 0:K])


        # ---- per-batch state ----
        st = []
        for lb in range(LB):
            st.append({
                k: work.tile(shp, dt, tag=k, name=f"{k}{lb}")
                for k, shp, dt in [
                    ("s_h0", [128, NT // 2], F32),
                    ("s_h1", [128, NT // 2], F32),
                    ("r_h0", [128, NT // 2], F32),
                    ("r_h1", [128, NT // 2], F32),
                    ("e_sb", [128, NT, K], F16),
                    ("et_sb", [128, NT, K], F16),
                    ("dl_sb", [128, NT, K], F8),
                    ("esum", [128, NT], F32),
                    ("recip", [128, NT], F32),
                    ("rtmp", [128, NT], F32),
                    ("rsc16", [128, NT], F16),
                    ("nrecipS", [128, NT], F16),
                ]
            })

        def phase_A(lb):
            """grams + diag extract + rsqrt halves + mm1 (half-interleaved)."""
            S = st[lb]
            gt_sb = gt_sbs[lb]
            gtp = gt_sb.rearrange("p (i two) n -> p i two n", two=2)
            wtp = wt_sb.rearrange("p (i two) k -> p i two k", two=2)
            zhs = []
            S["zhs"] = zhs
            for half in range(2):
                zps = ps_z.tile([128, NT // 2, K], F32, tag=f"z{half}",
                                name=f"z{lb}_{half}")
                zhs.append(zps)
                s_h = S[f"s_h{half}"]
                r_h = S[f"r_h{half}"]
                waves = [half * (NT // GW // 2) + w for w in range(NT // GW // 2)]
                for w in waves:
                    gps = ps_gram.tile([128, GW, 128], F32, tag="gram",
                                       name=f"gram{lb}_{w}")
                    for ti in range(GW):
                        t = w * GW + ti
                        lt = gtp[:, :, :, t * 128:(t + 1) * 128]
                        for i in range(DC // 2):
                            nc.tensor.matmul(
                                gps[:, ti, :], lt[:, i], lt[:, i],
                                start=(i == 0), stop=(i == DC // 2 - 1),
                                perf_mode=PM.DoubleRow)
                    for ti in range(GW):
                        t = w * GW + ti
                        th = t - half * (NT // 2)
                        sc_t = scr.tile([128, 128], F32, tag="xscr")
                        nc.vector.scalar_tensor_tensor(
                            out=sc_t, in0=gps[:, ti, :], scalar=1024.0,
                            in1=identF, op0=ALU.mult, op1=ALU.mult,
                            accum_out=s_h[:, th:th + 1])
                _emit_rsqrt(nc, work, r_h, s_h, [128, NT // 2])
                for t in range(half * (NT // 2), (half + 1) * (NT // 2)):
                    for i in range(DC // 2):
                        nc.tensor.matmul(
                            zps[:, t - half * (NT // 2), :],
                            gtp[:, i, :, t * 128:(t + 1) * 128],
                            wtp[:, i], start=(i == 0), stop=(i == DC // 2 - 1),
                            perf_mode=PM.DoubleRow)

        def phase_B(lb):
            """softmax: exp, e~, group scalars, delta, pac."""
            S = st[lb]
            small = ps_small.tile([128, 512], F32, tag="small",
                                  name=f"small{lb}")
            S["small"] = small
            pac = small[0:K, 0:1]
            S["pac"] = pac
            for t in range(NT):
                zt = S["zhs"][t // (NT // 2)][:, t % (NT // 2), :]
                r_h = S[f"s_h{0}"]  # placeholder
                r_h = S[f"r_h{t // (NT // 2)}"]
                th = t % (NT // 2)
                nc.scalar.activation(
                    out=S["e_sb"][:, t, :], in_=zt,
                    func=ACTF.Exp, scale=r_h[:, th:th + 1])
            for g in range(NG):
                gs = slice(g * GRP, (g + 1) * GRP)
                for t in range(g * GRP, (g + 1) * GRP):
                    nc.vector.scalar_tensor_tensor(
                        out=S["et_sb"][:, t, :], in0=S["e_sb"][:, t, :],
                        scalar=1.0, in1=ebb16, op0=ALU.mult, op1=ALU.mult,
                        accum_out=S["esum"][:, t:t + 1])
                nc.vector.reciprocal(out=S["recip"][:, gs], in_=S["esum"][:, gs])
                half = (g * GRP) // (NT // 2)
                hs = slice(g * GRP - half * (NT // 2),
                           (g + 1) * GRP - half * (NT // 2))
                nc.vector.tensor_mul(S["rtmp"][:, gs], S["recip"][:, gs],
                                     S[f"r_h{half}"][:, hs])
                nc.vector.tensor_scalar_mul(S["rsc16"][:, gs], S["rtmp"][:, gs],
                                            2.0 ** 20)
                nc.vector.tensor_scalar_mul(S["nrecipS"][:, gs],
                                            S["recip"][:, gs], -(2.0 ** 9))
                for t in range(g * GRP, (g + 1) * GRP):
                    nc.vector.scalar_tensor_tensor(
                        out=S["dl_sb"][:, t, :], in0=S["et_sb"][:, t, :],
                        scalar=-1.0,
                        in1=S["rsc16"][:, t:t + 1].broadcast_to((128, K)),
                        op0=ALU.add, op1=ALU.mult)
                    nc.tensor.matmul(
                        pac, S["et_sb"][:, t, :], S["nrecipS"][:, t:t + 1],
                        start=(t == 0), stop=(t == NT - 1))

        def phase_M(lb, half):
            """mm2 DR pairs (gn-gated); h1's last pair closes the p1 group."""
            S = st[lb]
            if half == 0:
                p1 = ps_p1.tile([128, DC, K], F32, tag="p1", name=f"p1_{lb}")
                S["p1"] = p1
            p1 = S["p1"]
            dlp = S["dl_sb"].rearrange("p (pr two) k -> p pr two k", two=2)
            gnp = gn_sbs[lb].rearrange("p (pr two) d -> p pr two d", two=2)
            prs = range(half * (NT // 4), (half + 1) * (NT // 4))
            last_pr = NT // 2 - 1
            for pr in prs:
                for c in range(DC):
                    nc.tensor.matmul(
                        p1[:, c, :],
                        gnp[:, pr, :, c * 128:(c + 1) * 128],
                        dlp[:, pr],
                        start=(pr == 0 and c == 0),
                        stop=(pr == last_pr and c == DC - 1),
                        perf_mode=PM.DoubleRow)

        def phase_C(lb):
            """v + broadcast + cent correction (closes the p1 group)."""
            S = st[lb]
            small = S["small"]
            p1 = S["p1"]
            v_ps = small[:, 8:8 + DC]
            gn_sb = gn_sbs[lb]
            for c in range(DC):
                for t in range(NT):
                    nc.tensor.matmul(
                        v_ps[:, c:c + 1],
                        gn_sb[:, t, c * 128:(c + 1) * 128],
                        S["rsc16"][:, t:t + 1],
                        start=(t == 0), stop=(t == NT - 1))
            v16 = work.tile([128, DC], F16, tag="v16")
            nc.vector.tensor_copy(out=v16, in_=v_ps)
            for c in range(DC):
                nc.tensor.matmul(
                    p1[:, c, :], ident16[:, :],
                    v16[:, c:c + 1].broadcast_to((128, K)),
                    start=False, stop=False)
            D16 = work.tile([K, K], F16, tag="D16")
            nc.vector.tensor_scalar(
                out=D16, in0=ident64, scalar1=S["pac"], scalar2=None,
                op0=ALU.mult)
            for c in range(DC):
                nc.tensor.matmul(
                    p1[:, c, :], cent_sb[:, c * 128:(c + 1) * 128], D16,
                    start=False, stop=False)

        def phase_D(lb):
            """finalize: qq, n2, rn, broadcast, scale, out DMA."""
            S = st[lb]
            small = S["small"]
            p1 = S["p1"]
            n2_ps = small[0:K, 16 + DC:16 + DC + 1]
            rnb_ps = small[:, 128:128 + K]
            qq16 = work.tile([128, DC, K], F16, tag="qq")
            nc.scalar.activation(out=qq16, in_=p1,
                                 func=ACTF.Square, scale=2.0 ** -10)
            for c in range(DC):
                nc.tensor.matmul(
                    n2_ps, qq16[:, c, :], ones128[:, 0:1],
                    start=(c == 0), stop=(c == DC - 1))
            n2_sb = work.tile([K, 1], F32, tag="n2sb")
            nc.vector.tensor_copy(out=n2_sb, in_=n2_ps)
            rn_sb = work.tile([K, 1], F32, tag="rn")
            _emit_rsqrt(nc, work, rn_sb, n2_sb, [K, 1], iters=1, tag="rsq2")
            rnD = work.tile([K, K], F16, tag="rnD")
            nc.vector.tensor_scalar(
                out=rnD, in0=ident64, scalar1=rn_sb, scalar2=None, op0=ALU.mult)
            nc.tensor.matmul(rnb_ps, ones64p[:, :], rnD[:, :],
                             start=True, stop=True)
            rnb16 = work.tile([128, K], F16, tag="rnb16")
            nc.vector.tensor_copy(out=rnb16, in_=rnb_ps)
            out_sb = work.tile([128, DC, K], F16, tag="out_sb")
            hc = DC // 2
            for h in range(2):
                cs = slice(h * hc, (h + 1) * hc)
                nc.vector.scalar_tensor_tensor(
                    out=out_sb[:, cs, :], in0=p1[:, cs, :], scalar=2.0 ** -10,
                    in1=rnb16.unsqueeze(1).broadcast_to((128, hc, K)),
                    op0=ALU.mult, op1=ALU.mult)
                nc.sync.dma_start(out=out_d[lb, :, cs], in_=out_sb[:, cs, :])

        phase_A(0)
        phase_B(0)
        phase_A(1)
        phase_M(0, 0)
        phase_B(1)
        phase_C(0)
        phase_M(0, 1)
        phase_D(0)
        phase_M(1, 0)
        phase_C(1)
        phase_M(1, 1)
        phase_D(1)

    nc.compile()
    return nc


def _get_nc():
    if "nc" not in _CACHE:
        _CACHE["nc"] = _gen_nc()
    return _CACHE["nc"]


def _prep_core_inputs(grids, W, b, centroids):
    """Host-side prep: fp8/fp16 casts + per-core sharded layouts."""
    import ml_dtypes

    f8 = ml_dtypes.float8_e4m3fn
    gn = np.ascontiguousarray(
        grids.astype(f8).reshape(B, NT, 128, D).transpose(0, 2, 1, 3))
    gt = np.ascontiguousarray(
        grids.transpose(0, 2, 1)).astype(f8).reshape(B, DC, 128, N)
    wt = np.ascontiguousarray((W * 32.0).T.astype(f8)).reshape(DC, 128, K)
    bias = b.astype(np.float32).reshape(1, K)
    cent = (centroids * 64.0).astype(np.float16).reshape(K, D)

    in_maps = []
    for c in range(N_CORES):
        sl = slice(c * LB, (c + 1) * LB)
        in_maps.append(
            {
                "gt": np.ascontiguousarray(gt[sl]),
                "gn": np.ascontiguousarray(gn[sl]),
                "wt": wt,
                "bias": bias,
                "cent": cent,
            }
        )
    return in_maps


def kernel(idx, grids, W, b, centroids):
    from concourse.bass_utils import run_bass_kernel_spmd

    nc = _get_nc()
    in_maps = _prep_core_inputs(
        np.asarray(grids, dtype=np.float32),
        np.asarray(W, dtype=np.float32),
        np.asarray(b, dtype=np.float32),
        np.asarray(centroids, dtype=np.float32),
    )
    res = run_bass_kernel_spmd(nc, in_maps, core_ids=list(range(N_CORES)))
    outs = []
    for c in range(N_CORES):
        o = res.results[c]["out"]                  # [LB, 128, DC, K] f16
        o = np.transpose(o.astype(np.float32), (0, 3, 2, 1))  # [LB, K, DC, 128]
        outs.append(o.reshape(LB, K, D))
    return np.concatenate(outs, axis=0)

